# revision 7
# baseline (speedup 1.0000x reference)
"""Trainium2 Bass kernel for CorrelatedSphericalField sampling.

Math (validated against the jax reference):
  coeffs[t] = PHI^t * d_t,   d_t = d_{t-1} + PHI^{-t} * sigma_n (.) xi_{t-1},  d_0 = coeff0
  xs[t,n,k,m] = sum_l coeffs[t,n,l,m] * pct[m,l,k]
  out[t,n,k,j] = 4pi * irfft_j(xs), done as two half-spectrum GEMMs:
      A[.., j] = sum_m xs_re[.., m] C[m, j],  B[.., j] = sum_m xs_im[.., m] S[m, j]
      out[.., 0:362] = A + B ;  out[.., 362+jj] = (A - B)[.., 360-jj]
  The PHI^t and 4pi factors are folded into per-core C/S constants.

Distribution (8 cores, single launch):
  stages A+B sharded over m (46 of 368 zero-padded m's per core, all (t,n)),
  AllToAll of xs (shard dim = t), stage D sharded over t (core c handles t=c).
"""
import numpy as np

import concourse.bass as bass
import concourse.mybir as mybir
import concourse.tile as tile
from concourse.bass_utils import run_bass_kernel_spmd

# ---- problem constants (hardcoded; kernel must be self-contained) ----
T = 8
N = 16
L = 361          # number of degrees l (contraction dim of stage B)
KLAT = 361       # number of latitudes
M = 362          # number of orders m
NLON = 722
JH = 362         # half-spectrum output columns of stage D
NC = 8
MPAD = 368       # M padded to a multiple of NC
MC = MPAD // NC  # 46 m's per core
TN = T * N       # 128
E = 2
NME = N * MC * E  # 1472
ME = MC * E       # 92

PHI = float(np.exp(-6.0 / 48.0))
FOUR_PI = float(4.0 * np.pi)

LCH = [(0, 128), (128, 256), (256, 361)]
MCH = [(0, 128), (128, 256), (256, 368)]
KCH = [(0, 128), (128, 256), (256, 361)]

F32 = mybir.dt.float32


def _mseg(a, b):
    """Split global-m range [a,b) into (core, mlo, count, poff) segments at MC boundaries."""
    segs = []
    start = a
    while start < b:
        c = start // MC
        hi = min(b, (c + 1) * MC)
        segs.append((c, start - c * MC, hi - start, start - a))
        start = hi
    return segs


def _split_multi_waits(nc, max_inline=1):
    """The walrus build in this env accepts only one inline sync-wait per
    instruction; hoist extras onto same-engine NoOps placed just before."""
    ctr = 0
    for f in nc.m.functions:
        for bb in f.blocks:
            new = []
            for inst in bb.instructions:
                si = inst.sync_info
                if si is not None and si.on_wait and len(si.on_wait) > max_inline:
                    waits = list(si.on_wait)
                    keep = waits[-max_inline:]
                    for w in waits[:-max_inline]:
                        ctr += 1
                        nop = mybir.InstNoOp(name=f"I-wsplit-{ctr}",
                                             engine=inst.engine)
                        nop.sync_info = mybir.SyncInfo(on_wait=[w], on_update=[])
                        new.append(nop)
                    inst.sync_info = mybir.SyncInfo(
                        on_wait=keep, on_update=list(si.on_update))
                new.append(inst)
            bb.instructions = new


def build_nc(split_waits=True):
    nc = bass.Bass(num_devices=NC)

    xi_p = nc.declare_dram_parameter("xi_t", [L, T, NME], F32, isOutput=False)
    c0_p = nc.declare_dram_parameter("c0_t", [L, NME], F32, isOutput=False)
    sig_p = nc.declare_dram_parameter("sigp", [L, T, ME], F32, isOutput=False)
    pct_p = nc.declare_dram_parameter("pct_t", [MC, L, KLAT], F32, isOutput=False)
    csC_p = nc.declare_dram_parameter("csC", [MPAD, JH], F32, isOutput=False)
    csS_p = nc.declare_dram_parameter("csS", [MPAD, JH], F32, isOutput=False)
    out_p = nc.declare_dram_parameter("out_t", [N, KLAT, NLON], F32, isOutput=True)

    with tile.TileContext(nc) as tc:
        with tc.tile_pool(name="dram", bufs=1, space="DRAM") as pdram:
            xs_send = pdram.tile([TN, E, MC, KLAT], F32, tag="xs_send")
            xs_recv = pdram.tile([TN, E, MC, KLAT], F32, tag="xs_recv")

            # ---------------- stage A: AR(1) prefix (d-space) -------------------
            with tc.tile_pool(name="per", bufs=1) as pa:
                d_tiles = []
                sig_tiles = []
                for lc, (la, lb) in enumerate(LCH):
                    lp = lb - la
                    dt_ = pa.tile([lp, T, NME], F32, tag=f"d{lc}")
                    st_ = pa.tile([lp, T, ME], F32, tag=f"sig{lc}")
                    nc.sync.dma_start(st_[:], sig_p[la:lb])
                    nc.sync.dma_start(dt_[:, 0], c0_p[la:lb])
                    d_tiles.append(dt_)
                    sig_tiles.append(st_)

                with tc.tile_pool(name="xi", bufs=3) as px:
                    for t in range(1, T):
                        for lc, (la, lb) in enumerate(LCH):
                            lp = lb - la
                            xi_sb = px.tile([lp, NME], F32, tag="xi")
                            z_sb = px.tile([lp, NME], F32, tag="z")
                            nc.sync.dma_start(xi_sb[:], xi_p[la:lb, t - 1])
                            sig_b = sig_tiles[lc][:, t - 1][:, None, :].broadcast_to(
                                [lp, N, ME])
                            nc.vector.tensor_tensor(
                                out=z_sb[:].rearrange("p (n q) -> p n q", q=ME),
                                in0=xi_sb[:].rearrange("p (n q) -> p n q", q=ME),
                                in1=sig_b, op=mybir.AluOpType.mult)
                            nc.vector.tensor_tensor(
                                out=d_tiles[lc][:, t], in0=d_tiles[lc][:, t - 1],
                                in1=z_sb[:], op=mybir.AluOpType.add)

                # ---------------- stage B: per-m Legendre GEMM ------------------
                with (
                    tc.tile_pool(name="w", bufs=2) as pw,
                    tc.tile_pool(name="xs", bufs=4) as pxs,
                    tc.tile_pool(name="psB", bufs=2, space="PSUM") as pp,
                ):
                    for m in range(MC):
                        pcts = []
                        for lc, (la, lb) in enumerate(LCH):
                            lp = lb - la
                            w = pw.tile([lp, KLAT], F32, tag=f"pct{lc}")
                            nc.sync.dma_start(w[:], pct_p[m, la:lb])
                            pcts.append(w)
                        for e in range(E):
                            ps = pp.tile([TN, KLAT], F32, tag=f"ps{e}")
                            for lc in range(3):
                                # lhsT: [l, t, n] at fixed (m, e) via strided slice
                                lhsT = d_tiles[lc][:, :, m * E + e:NME:ME]
                                nc.tensor.matmul(
                                    ps[:], lhsT, pcts[lc][:],
                                    start=(lc == 0), stop=(lc == 2))
                            xs_sb = pxs.tile([TN, KLAT], F32, tag=f"xsb{e}")
                            if e == 0:
                                nc.scalar.copy(xs_sb[:], ps[:])
                            else:
                                nc.vector.tensor_copy(xs_sb[:], ps[:])
                            nc.sync.dma_start(xs_send[:, e, m], xs_sb[:])

            # ---------------- stage C: reshard m -> t ---------------------------
            nc.gpsimd.collective_compute(
                "AllToAll", mybir.AluOpType.bypass,
                replica_groups=[list(range(NC))],
                ins=[xs_send.opt()], outs=[xs_recv.opt()])

            # ---------------- stage D: iFFT GEMM over m -------------------------
            with (
                tc.tile_pool(name="cs", bufs=1) as pcs,
                tc.tile_pool(name="xr", bufs=2) as pxr,
                tc.tile_pool(name="o", bufs=3) as po,
                tc.tile_pool(name="psD", bufs=2, space="PSUM") as pp2,
            ):
                csC_t, csS_t = [], []
                for mc, (ma, mb) in enumerate(MCH):
                    mp = mb - ma
                    ct = pcs.tile([mp, JH], F32, tag=f"csC{mc}")
                    st = pcs.tile([mp, JH], F32, tag=f"csS{mc}")
                    nc.sync.dma_start(ct[:], csC_p[ma:mb])
                    nc.sync.dma_start(st[:], csS_p[ma:mb])
                    csC_t.append(ct)
                    csS_t.append(st)

                for n in range(N):
                    xr = {}
                    for e in range(E):
                        for mc, (ma, mb) in enumerate(MCH):
                            mp = mb - ma
                            xt = pxr.tile([mp, KLAT], F32, tag=f"xr{e}{mc}")
                            for (c, mlo, cnt, poff) in _mseg(ma, mb):
                                nc.sync.dma_start(
                                    xt[poff:poff + cnt],
                                    xs_recv[16 * c + n, e, mlo:mlo + cnt])
                            xr[(e, mc)] = xt
                    for (ka, kb) in KCH:
                        kp = kb - ka
                        psA = pp2.tile([kp, JH], F32, tag="psA")
                        psB = pp2.tile([kp, JH], F32, tag="psB")
                        for mc in range(3):
                            nc.tensor.matmul(
                                psA[:], xr[(0, mc)][:, ka:kb], csC_t[mc][:],
                                start=(mc == 0), stop=(mc == 2))
                        for mc in range(3):
                            nc.tensor.matmul(
                                psB[:], xr[(1, mc)][:, ka:kb], csS_t[mc][:],
                                start=(mc == 0), stop=(mc == 2))
                        a_sb = po.tile([kp, JH], F32, tag="a_sb")
                        o1 = po.tile([kp, JH], F32, tag="o1")
                        o2 = po.tile([kp, JH - 2], F32, tag="o2")
                        nc.scalar.copy(a_sb[:], psA[:])
                        nc.vector.tensor_tensor(
                            out=o1[:], in0=a_sb[:], in1=psB[:],
                            op=mybir.AluOpType.add)
                        nc.vector.tensor_tensor(
                            out=o2[:], in0=a_sb[:, JH - 2:0:-1],
                            in1=psB[:, JH - 2:0:-1],
                            op=mybir.AluOpType.subtract)
                        nc.sync.dma_start(out_p[n, ka:kb, 0:JH], o1[:])
                        nc.sync.dma_start(out_p[n, ka:kb, JH:NLON], o2[:])

    if split_waits:
        _split_multi_waits(nc)
    return nc


def prep_inputs(x, sigma_n, coeff0, xi, pct):
    """Host-side shard/stage: slice + transpose per-core inputs, build constants."""
    sigma_n = np.asarray(sigma_n, np.float32)
    coeff0 = np.asarray(coeff0, np.float32)
    xi = np.asarray(xi, np.float32)
    pct = np.asarray(pct, np.float32)

    padm = MPAD - M
    sig_pad = np.pad(sigma_n, ((0, 0), (0, padm)))
    c0_pad = np.pad(coeff0, ((0, 0), (0, 0), (0, padm), (0, 0)))
    xi_pad = np.pad(xi, ((0, 0), (0, 0), (0, 0), (0, padm), (0, 0)))
    pct_pad = np.pad(pct, ((0, padm), (0, 0), (0, 0)))

    # half-spectrum irfft matrices (fp64 host build)
    j = np.arange(JH, dtype=np.float64)
    mm = np.arange(M, dtype=np.float64)
    ang = 2.0 * np.pi * np.outer(mm, j) / NLON
    C = 2.0 * np.cos(ang)
    C[0, :] = 1.0
    C[M - 1, :] = np.cos(np.pi * j)
    S = -2.0 * np.sin(ang)
    S[0, :] = 0.0
    S[M - 1, :] = 0.0
    Cp = np.pad(C, ((0, padm), (0, 0)))
    Sp = np.pad(S, ((0, padm), (0, 0)))

    phi_inv = PHI ** -(np.arange(T, dtype=np.float64) + 1.0)

    in_maps = []
    for c in range(NC):
        msl = slice(c * MC, (c + 1) * MC)
        xi_c = np.ascontiguousarray(
            np.transpose(xi_pad[:, :, :, msl, :], (2, 0, 1, 3, 4))).reshape(L, T, NME)
        c0_c = np.ascontiguousarray(
            np.transpose(c0_pad[:, :, msl, :], (1, 0, 2, 3))).reshape(L, NME)
        # sigma' expanded over the re/im dim: [L, T, MC*E]
        sig_c = (sig_pad[:, msl][:, None, :] * phi_inv[None, :, None])
        sig_c = np.repeat(sig_c[:, :, :, None], E, axis=3).reshape(L, T, ME)
        pct_c = np.ascontiguousarray(pct_pad[msl])
        scale = FOUR_PI * PHI ** c
        in_maps.append({
            "xi_t": xi_c,
            "c0_t": c0_c,
            "sigp": np.ascontiguousarray(sig_c).astype(np.float32),
            "pct_t": pct_c,
            "csC": (scale * Cp).astype(np.float32),
            "csS": (scale * Sp).astype(np.float32),
        })
    return in_maps


_NC_CACHE = None


def kernel(x, sigma_n, coeff0, xi, pct):
    global _NC_CACHE
    in_maps = prep_inputs(x, sigma_n, coeff0, xi, pct)
    if _NC_CACHE is None:
        _NC_CACHE = build_nc()
    res = run_bass_kernel_spmd(_NC_CACHE, in_maps, list(range(NC)))
    out = np.stack([res.results[c]["out_t"] for c in range(NC)], axis=0)
    return out.reshape(T, 1, 1, N, KLAT, NLON)


# revision 10
# speedup vs baseline: 1.8209x; 1.8209x over previous
"""Trainium2 Bass kernel for CorrelatedSphericalField sampling (bf16 v2).

Math (validated against the jax reference):
  coeffs[t] = PHI^t * d_t,   d_t = d_{t-1} + PHI^{-t} * sigma_n (.) xi_{t-1},  d_0 = coeff0
  xs[t,n,k,m] = sum_l d[t,n,l,m] * pct[m,l,k]          (per-m Legendre GEMM)
  out[t,n,k,j] = 4pi * PHI^t * irfft_j(xs), as half-spectrum GEMMs:
      A[.., j] = sum_m xs_re[.., m] C[m, j],  B[.., j] = sum_m xs_im[.., m] S[m, j]
      out[.., 0:362] = A + B ;  out[.., 362+jj] = (A - B)[.., 360-jj]
  PHI^t and 4pi are folded into per-core C/S constants.

Distribution (8 cores, single launch):
  stages A+B sharded over m (46 of 368 zero-padded m's per core, all (t,n)),
  AllToAll of xs in 4 m-chunks overlapped with stage B (shard dim = t),
  stage D sharded over t (core c handles t=c).

Data is bf16 end to end (fp32 PSUM accumulation, fp32 output).
"""
import numpy as np
import ml_dtypes

import concourse.bass as bass
import concourse.mybir as mybir
import concourse.tile as tile
from concourse.bass_utils import run_bass_kernel_spmd

# ---- problem constants (hardcoded; kernel must be self-contained) ----
T = 8
N = 16
L = 361          # number of degrees l (contraction dim of stage B)
L2 = 384         # L zero-padded to 3*128
KLAT = 361       # number of latitudes
M = 362          # number of orders m
NLON = 722
JH = 362         # half-spectrum output columns of stage D
NC = 8
MPAD = 368       # M padded to a multiple of NC
MC = MPAD // NC  # 46 m's per core
TN = T * N       # 128
E = 2
ME = MC * E      # 92
MEN = ME * N     # 1472

PHI = float(np.exp(-6.0 / 48.0))
FOUR_PI = float(4.0 * np.pi)

LCH = [(0, 128), (128, 256), (256, 384)]
MCH = [(0, 128), (128, 256), (256, 368)]
KCH = [(0, 128), (128, 256), (256, 361)]
MGRP = [(0, 12), (12, 24), (24, 36), (36, 46)]   # A2A m-chunks within a core

F32 = mybir.dt.float32
BF16 = mybir.dt.bfloat16
NPBF = ml_dtypes.bfloat16


def _dseg(a, b):
    """Split global-m [a,b) into (core, group, local-in-group m0, count, poff)
    segments at core (MC) and A2A-group boundaries."""
    segs = []
    mg = a
    while mg < b:
        c = mg // MC
        local = mg - c * MC
        g = next(i for i, (ga, gb) in enumerate(MGRP) if ga <= local < gb)
        hi = min(b, c * MC + MGRP[g][1])
        segs.append((c, g, local - MGRP[g][0], hi - mg, mg - a))
        mg = hi
    return segs


def _split_multi_waits(nc, max_inline=1):
    """The walrus build in this env accepts only one inline sync-wait per
    instruction; hoist extras onto same-engine NoOps placed just before."""
    ctr = 0
    for f in nc.m.functions:
        for bb in f.blocks:
            new = []
            for inst in bb.instructions:
                si = inst.sync_info
                if si is not None and si.on_wait and len(si.on_wait) > max_inline:
                    waits = list(si.on_wait)
                    keep = waits[-max_inline:]
                    for w in waits[:-max_inline]:
                        ctr += 1
                        nop = mybir.InstNoOp(name=f"I-wsplit-{ctr}",
                                             engine=inst.engine)
                        nop.sync_info = mybir.SyncInfo(on_wait=[w], on_update=[])
                        new.append(nop)
                    inst.sync_info = mybir.SyncInfo(
                        on_wait=keep, on_update=list(si.on_update))
                new.append(inst)
            bb.instructions = new


def build_nc(split_waits=True):
    nc = bass.Bass(num_devices=NC)

    # layouts: xi [l, t, (m,e,n)], c0 [l, (m,e,n)], sigp [l, t, (m,e)],
    # pct [m, l, k] (l padded to 384), csC/csS [mpad, jh]
    xi_p = nc.declare_dram_parameter("xi_t", [L2, T, MEN], BF16, isOutput=False)
    c0_p = nc.declare_dram_parameter("c0_t", [L2, MEN], BF16, isOutput=False)
    sig_p = nc.declare_dram_parameter("sigp", [L2, T, ME], BF16, isOutput=False)
    pct_p = nc.declare_dram_parameter("pct_t", [MC, L2, KLAT], BF16, isOutput=False)
    csC_p = nc.declare_dram_parameter("csC", [MPAD, JH], BF16, isOutput=False)
    csS_p = nc.declare_dram_parameter("csS", [MPAD, JH], BF16, isOutput=False)
    out_p = nc.declare_dram_parameter("out_t", [N, KLAT, NLON], F32, isOutput=True)

    with tile.TileContext(nc) as tc:
        with tc.tile_pool(name="dram", bufs=1, space="DRAM") as pdram:
            sends, recvs = [], []
            for g, (ga, gb) in enumerate(MGRP):
                mg = gb - ga
                sends.append(pdram.tile([TN, E, mg, KLAT], BF16,
                                        name=f"send{g}", tag=f"send{g}"))
                recvs.append(pdram.tile([TN, E, mg, KLAT], BF16,
                                        name=f"recv{g}", tag=f"recv{g}"))

            # ---------------- stage A: AR(1) prefix (d-space) ---------------
            with tc.tile_pool(name="per", bufs=1) as pa:
                d_tiles = []
                sig_tiles = []
                for lc, (la, lb) in enumerate(LCH):
                    dt_ = pa.tile([128, ME, T, N], BF16, tag=f"d{lc}")
                    st_ = pa.tile([128, T, ME], BF16, tag=f"sig{lc}")
                    nc.sync.dma_start(st_[:], sig_p[la:lb])
                    # d_0 = c0 : dst [l, (me), 0, (n)]
                    nc.sync.dma_start(
                        dt_[:, :, 0, :],
                        c0_p[la:lb].rearrange("p (q n) -> p q n", q=ME))
                    d_tiles.append(dt_)
                    sig_tiles.append(st_)

                with tc.tile_pool(name="xi", bufs=3) as px:
                    for t in range(1, T):
                        for lc, (la, lb) in enumerate(LCH):
                            xi_sb = px.tile([128, ME, N], BF16, tag="xi")
                            z_sb = px.tile([128, ME, N], BF16, tag="z")
                            nc.sync.dma_start(
                                xi_sb[:],
                                xi_p[la:lb, t - 1].rearrange(
                                    "p (q n) -> p q n", q=ME))
                            sig_b = sig_tiles[lc][:, t - 1][:, :, None].broadcast_to(
                                [128, ME, N])
                            nc.vector.tensor_tensor(
                                out=z_sb[:], in0=xi_sb[:], in1=sig_b,
                                op=mybir.AluOpType.mult)
                            nc.vector.tensor_tensor(
                                out=d_tiles[lc][:, :, t, :],
                                in0=d_tiles[lc][:, :, t - 1, :],
                                in1=z_sb[:], op=mybir.AluOpType.add)

                # ------------- stage B: per-m Legendre GEMM + chunked A2A ----
                with (
                    tc.tile_pool(name="w", bufs=2) as pw,
                    tc.tile_pool(name="xs", bufs=4) as pxs,
                    tc.tile_pool(name="psB", bufs=2, space="PSUM") as pp,
                ):
                    for g, (ga, gb) in enumerate(MGRP):
                        for m in range(ga, gb):
                            w = pw.tile([128, 3, KLAT], BF16, tag="pct")
                            nc.sync.dma_start(
                                w[:],
                                pct_p[m].rearrange("(c p) k -> p c k", p=128))
                            for e in range(E):
                                ps = pp.tile([TN, KLAT], F32, tag=f"ps{e}")
                                for lc in range(3):
                                    nc.tensor.matmul(
                                        ps[:], d_tiles[lc][:, m * E + e],
                                        w[:, lc],
                                        start=(lc == 0), stop=(lc == 2))
                                xs_sb = pxs.tile([TN, KLAT], BF16, tag=f"xsb{e}")
                                if e == 0:
                                    nc.scalar.copy(xs_sb[:], ps[:])
                                else:
                                    nc.vector.tensor_copy(xs_sb[:], ps[:])
                                nc.sync.dma_start(
                                    sends[g][:, e, m - ga], xs_sb[:])
                        nc.gpsimd.collective_compute(
                            "AllToAll", mybir.AluOpType.bypass,
                            replica_groups=[list(range(NC))],
                            ins=[sends[g].opt()], outs=[recvs[g].opt()])

            # ---------------- stage D: iFFT GEMM over m ---------------------
            with (
                tc.tile_pool(name="cs", bufs=1) as pcs,
                tc.tile_pool(name="xr", bufs=1) as pxr,
                tc.tile_pool(name="o", bufs=3) as po,
                tc.tile_pool(name="psD", bufs=2, space="PSUM") as pp2,
            ):
                csC_t, csS_t = [], []
                for mc, (ma, mb) in enumerate(MCH):
                    mp = mb - ma
                    ct = pcs.tile([mp, JH], BF16, tag=f"csC{mc}")
                    st = pcs.tile([mp, JH], BF16, tag=f"csS{mc}")
                    nc.sync.dma_start(ct[:], csC_p[ma:mb])
                    nc.sync.dma_start(st[:], csS_p[ma:mb])
                    csC_t.append(ct)
                    csS_t.append(st)

                # load the full resharded xs: [m | n*k] per (e, m-chunk)
                xr = {}
                for e in range(E):
                    for mc, (ma, mb) in enumerate(MCH):
                        mp = mb - ma
                        xt = pxr.tile([mp, N * KLAT], BF16, tag=f"xr{e}{mc}")
                        for (c, g, mlg, cnt, poff) in _dseg(ma, mb):
                            mg = MGRP[g][1] - MGRP[g][0]
                            src = recvs[g][16 * c:16 * (c + 1), e,
                                           mlg:mlg + cnt, :]
                            # [n, m, k] -> partition m, free (n, k)
                            nc.sync.dma_start(
                                xt[poff:poff + cnt].rearrange(
                                    "p (n k) -> p n k", n=N),
                                src.transpose([1, 0, 2]))
                        xr[(e, mc)] = xt

                for n in range(N):
                    for (ka, kb) in KCH:
                        kp = kb - ka
                        psA = pp2.tile([kp, JH], F32, tag="psA")
                        psB = pp2.tile([kp, JH], F32, tag="psB")
                        for mc in range(3):
                            nc.tensor.matmul(
                                psA[:],
                                xr[(0, mc)][:, n * KLAT + ka:n * KLAT + kb],
                                csC_t[mc][:],
                                start=(mc == 0), stop=(mc == 2))
                        for mc in range(3):
                            nc.tensor.matmul(
                                psB[:],
                                xr[(1, mc)][:, n * KLAT + ka:n * KLAT + kb],
                                csS_t[mc][:],
                                start=(mc == 0), stop=(mc == 2))
                        a_sb = po.tile([kp, JH], F32, tag="a_sb")
                        o1 = po.tile([kp, JH], F32, tag="o1")
                        o2 = po.tile([kp, JH - 2], F32, tag="o2")
                        nc.scalar.copy(a_sb[:], psA[:])
                        nc.vector.tensor_tensor(
                            out=o1[:], in0=a_sb[:], in1=psB[:],
                            op=mybir.AluOpType.add)
                        nc.vector.tensor_tensor(
                            out=o2[:], in0=a_sb[:, JH - 2:0:-1],
                            in1=psB[:, JH - 2:0:-1],
                            op=mybir.AluOpType.subtract)
                        nc.sync.dma_start(out_p[n, ka:kb, 0:JH], o1[:])
                        nc.sync.dma_start(out_p[n, ka:kb, JH:NLON], o2[:])

    if split_waits:
        _split_multi_waits(nc)
    return nc


def prep_inputs(x, sigma_n, coeff0, xi, pct):
    """Host-side shard/stage: slice + transpose per-core inputs, build constants."""
    sigma_n = np.asarray(sigma_n, np.float32)
    coeff0 = np.asarray(coeff0, np.float32)
    xi = np.asarray(xi, np.float32)
    pct = np.asarray(pct, np.float32)

    padm = MPAD - M
    padl = L2 - L
    sig_pad = np.pad(sigma_n, ((0, padl), (0, padm)))
    c0_pad = np.pad(coeff0, ((0, 0), (0, padl), (0, padm), (0, 0)))
    xi_pad = np.pad(xi, ((0, 0), (0, 0), (0, padl), (0, padm), (0, 0)))
    pct_pad = np.pad(pct, ((0, padm), (0, padl), (0, 0)))

    # half-spectrum irfft matrices (fp64 host build)
    j = np.arange(JH, dtype=np.float64)
    mm = np.arange(M, dtype=np.float64)
    ang = 2.0 * np.pi * np.outer(mm, j) / NLON
    C = 2.0 * np.cos(ang)
    C[0, :] = 1.0
    C[M - 1, :] = np.cos(np.pi * j)
    S = -2.0 * np.sin(ang)
    S[0, :] = 0.0
    S[M - 1, :] = 0.0
    Cp = np.pad(C, ((0, padm), (0, 0)))
    Sp = np.pad(S, ((0, padm), (0, 0)))

    phi_inv = PHI ** -(np.arange(T, dtype=np.float64) + 1.0)

    in_maps = []
    for c in range(NC):
        msl = slice(c * MC, (c + 1) * MC)
        # xi: [t,n,l,m,e] -> [l,t,m,e,n]
        xi_c = np.ascontiguousarray(
            np.transpose(xi_pad[:, :, :, msl, :], (2, 0, 3, 4, 1))
        ).reshape(L2, T, MEN).astype(NPBF)
        # c0: [n,l,m,e] -> [l,m,e,n]
        c0_c = np.ascontiguousarray(
            np.transpose(c0_pad[:, :, msl, :], (1, 2, 3, 0))
        ).reshape(L2, MEN).astype(NPBF)
        sig_c = (sig_pad[:, msl][:, None, :] * phi_inv[None, :, None])
        sig_c = np.repeat(sig_c[:, :, :, None], E, axis=3).reshape(L2, T, ME)
        pct_c = np.ascontiguousarray(pct_pad[msl]).astype(NPBF)
        scale = FOUR_PI * PHI ** c
        in_maps.append({
            "xi_t": xi_c,
            "c0_t": c0_c,
            "sigp": np.ascontiguousarray(sig_c).astype(NPBF),
            "pct_t": pct_c,
            "csC": (scale * Cp).astype(NPBF),
            "csS": (scale * Sp).astype(NPBF),
        })
    return in_maps


_NC_CACHE = None


def kernel(x, sigma_n, coeff0, xi, pct):
    global _NC_CACHE
    in_maps = prep_inputs(x, sigma_n, coeff0, xi, pct)
    if _NC_CACHE is None:
        _NC_CACHE = build_nc()
    res = run_bass_kernel_spmd(_NC_CACHE, in_maps, list(range(NC)))
    out = np.stack([res.results[c]["out_t"] for c in range(NC)], axis=0)
    return out.reshape(T, 1, 1, N, KLAT, NLON)


# revision 13
# speedup vs baseline: 1.8787x; 1.0318x over previous
"""Trainium2 Bass kernel for CorrelatedSphericalField sampling (bf16 v3).

Math (validated against the jax reference):
  coeffs[t] = PHI^t * d_t,   d_t = d_{t-1} + PHI^{-t} * sigma_n (.) xi_{t-1},  d_0 = coeff0
  xs[t,n,k,m] = sum_l d[t,n,l,m] * pct[m,l,k]          (per-m Legendre GEMM)
  out[t,n,k,j] = 4pi * PHI^t * irfft_j(xs), as half-spectrum GEMMs:
      A[.., j] = sum_m xs_re[.., m] C[m, j],  B[.., j] = sum_m xs_im[.., m] S[m, j]
      out[.., 0:362] = A + B ;  out[.., 362+jj] = (A - B)[.., 360-jj]
  PHI^t and 4pi are folded into per-core C/S constants.

Distribution (8 cores, single launch):
  stages A+B sharded over m (46 of 368 zero-padded m's per core, all (t,n)),
  processed in 4 m-groups pipelined with a chunked AllToAll of xs (shard dim
  = t); stage D sharded over t (core c handles t=c).

Data is bf16 end to end (fp32 PSUM accumulation, fp32 output).
"""
import numpy as np
import ml_dtypes

import concourse.bass as bass
import concourse.mybir as mybir
import concourse.tile as tile
from concourse.bass_utils import run_bass_kernel_spmd

# ---- problem constants (hardcoded; kernel must be self-contained) ----
T = 8
N = 16
L = 361          # number of degrees l (contraction dim of stage B)
L2 = 384         # L zero-padded to 3*128
KLAT = 361       # number of latitudes
M = 362          # number of orders m
NLON = 722
JH = 362         # half-spectrum output columns of stage D
NC = 8
MPAD = 368       # M padded to a multiple of NC
MC = MPAD // NC  # 46 m's per core
TN = T * N       # 128
E = 2

PHI = float(np.exp(-6.0 / 48.0))
FOUR_PI = float(4.0 * np.pi)

LCH = [(0, 128), (128, 256), (256, 384)]
MCH = [(0, 128), (128, 256), (256, 368)]
KCH = [(0, 128), (128, 256), (256, 361)]
MGRP = [(0, 12), (12, 24), (24, 36), (36, 46)]   # A2A m-chunks within a core
G = len(MGRP)
GM = 12                  # per-group m slots (padded; last group uses 10)
GME = GM * E             # 24
GMEN = GME * N           # 384
MENP = G * GMEN          # 1536 (padded (g, m, e, n) free size)

F32 = mybir.dt.float32
BF16 = mybir.dt.bfloat16
NPBF = ml_dtypes.bfloat16


def _dseg(a, b):
    """Split global-m [a,b) into (core, group, local-in-group m0, count, poff)
    segments at core (MC) and A2A-group boundaries."""
    segs = []
    mg = a
    while mg < b:
        c = mg // MC
        local = mg - c * MC
        g = next(i for i, (ga, gb) in enumerate(MGRP) if ga <= local < gb)
        hi = min(b, c * MC + MGRP[g][1])
        segs.append((c, g, local - MGRP[g][0], hi - mg, mg - a))
        mg = hi
    return segs


def _split_multi_waits(nc, max_inline=1):
    """The walrus build in this env accepts only one inline sync-wait per
    instruction; hoist extras onto same-engine NoOps placed just before."""
    ctr = 0
    for f in nc.m.functions:
        for bb in f.blocks:
            new = []
            for inst in bb.instructions:
                si = inst.sync_info
                if si is not None and si.on_wait and len(si.on_wait) > max_inline:
                    waits = list(si.on_wait)
                    keep = waits[-max_inline:]
                    for w in waits[:-max_inline]:
                        ctr += 1
                        nop = mybir.InstNoOp(name=f"I-wsplit-{ctr}",
                                             engine=inst.engine)
                        nop.sync_info = mybir.SyncInfo(on_wait=[w], on_update=[])
                        new.append(nop)
                    inst.sync_info = mybir.SyncInfo(
                        on_wait=keep, on_update=list(si.on_update))
                new.append(inst)
            bb.instructions = new


def build_nc(split_waits=True):
    nc = bass.Bass(num_devices=NC)

    # host layouts (zero-padded into G groups of GM m-slots):
    #   xi  [l, t, (g, m, e, n)]       c0 [l, (g, m, e, n)]
    #   sig [l, g, t, (m, e)]          pct [m, l(384), k]
    xi_p = nc.declare_dram_parameter("xi_t", [L2, T, MENP], BF16, isOutput=False)
    c0_p = nc.declare_dram_parameter("c0_t", [L2, MENP], BF16, isOutput=False)
    sig_p = nc.declare_dram_parameter("sigp", [L2, G, T, GME], BF16, isOutput=False)
    pct_p = nc.declare_dram_parameter("pct_t", [MC, L2, KLAT], BF16, isOutput=False)
    csC_p = nc.declare_dram_parameter("csC", [MPAD, JH], BF16, isOutput=False)
    csS_p = nc.declare_dram_parameter("csS", [MPAD, JH], BF16, isOutput=False)
    out_p = nc.declare_dram_parameter("out_t", [N, KLAT, NLON], F32, isOutput=True)

    with tile.TileContext(nc) as tc:
        with tc.tile_pool(name="dram", bufs=1, space="DRAM") as pdram:
            sends, recvs = [], []
            for g, (ga, gb) in enumerate(MGRP):
                mg = gb - ga
                sends.append(pdram.tile([TN, E, mg, KLAT], BF16,
                                        name=f"send{g}", tag=f"send{g}"))
                recvs.append(pdram.tile([TN, E, mg, KLAT], BF16,
                                        name=f"recv{g}", tag=f"recv{g}"))

            with (
                tc.tile_pool(name="per", bufs=1) as pa,
                tc.tile_pool(name="xi", bufs=3) as px,
                tc.tile_pool(name="w", bufs=3) as pw,
                tc.tile_pool(name="xs", bufs=4) as pxs,
                tc.tile_pool(name="psB", bufs=2, space="PSUM") as pp,
            ):
                # persistent: sigma', c0 staging, per-(lc,g) d tiles
                sig_tiles, c0_tiles = [], []
                for lc, (la, lb) in enumerate(LCH):
                    st_ = pa.tile([128, G, T, GME], BF16, tag=f"sig{lc}")
                    ct_ = pa.tile([128, MENP], BF16, tag=f"c0{lc}")
                    nc.sync.dma_start(st_[:], sig_p[la:lb])
                    nc.sync.dma_start(ct_[:], c0_p[la:lb])
                    sig_tiles.append(st_)
                    c0_tiles.append(ct_)
                d_tiles = {}
                for g in range(G):
                    for lc in range(3):
                        d_tiles[(lc, g)] = pa.tile([128, GME, T, N], BF16,
                                                   name=f"d{lc}g{g}",
                                                   tag=f"d{lc}g{g}")

                for g, (ga, gb) in enumerate(MGRP):
                    # ---- stage A for group g: z = sigma' (.) xi; d prefix-sum
                    for lc, (la, lb) in enumerate(LCH):
                        dt_ = d_tiles[(lc, g)]
                        xi_sb = px.tile([128, T * GMEN], BF16, tag="xi")
                        z_sb = px.tile([128, T * GME, N], BF16, tag="z")
                        nc.sync.dma_start(
                            xi_sb[:].rearrange("p (t q) -> p t q", t=T),
                            xi_p[la:lb, :, g * GMEN:(g + 1) * GMEN])
                        sig_b = sig_tiles[lc][:, g].rearrange(
                            "p t q -> p (t q)")[:, :, None].broadcast_to(
                                [128, T * GME, N])
                        nc.vector.tensor_tensor(
                            out=z_sb[:],
                            in0=xi_sb[:].rearrange("p (tq n) -> p tq n", n=N),
                            in1=sig_b,
                            op=mybir.AluOpType.mult)
                        # d_0 = c0 (copy from contiguous staging)
                        nc.vector.tensor_copy(
                            dt_[:, :, 0, :],
                            c0_tiles[lc][:, g * GMEN:(g + 1) * GMEN].rearrange(
                                "p (q n) -> p q n", n=N))
                        for t in range(1, T):
                            nc.vector.tensor_tensor(
                                out=dt_[:, :, t, :],
                                in0=dt_[:, :, t - 1, :],
                                in1=z_sb[:, (t - 1) * GME:t * GME, :],
                                op=mybir.AluOpType.add)

                    # ---- stage B for group g: per-m Legendre GEMM -----------
                    for mp_ in range(ga, gb, 2):
                        npair = min(2, gb - mp_)
                        w = pw.tile([128, npair, 3, KLAT], BF16, tag="pct")
                        nc.sync.dma_start(
                            w[:],
                            pct_p[mp_:mp_ + npair].rearrange(
                                "m (c p) k -> p m c k", p=128))
                        for mi in range(npair):
                            m = mp_ + mi
                            gm = m - ga          # m-slot within group
                            xs_sb = pxs.tile([TN, E, KLAT], BF16, tag="xsb")
                            for e in range(E):
                                ps = pp.tile([TN, KLAT], F32, tag=f"ps{e}")
                                for lc in range(3):
                                    nc.tensor.matmul(
                                        ps[:],
                                        d_tiles[(lc, g)][:, gm * E + e],
                                        w[:, mi, lc],
                                        start=(lc == 0), stop=(lc == 2))
                                if e == 0:
                                    nc.scalar.copy(xs_sb[:, 0], ps[:])
                                else:
                                    nc.vector.tensor_copy(xs_sb[:, 1], ps[:])
                            nc.sync.dma_start(sends[g][:, :, gm], xs_sb[:])

                    nc.gpsimd.collective_compute(
                        "AllToAll", mybir.AluOpType.bypass,
                        replica_groups=[list(range(NC))],
                        ins=[sends[g].opt()], outs=[recvs[g].opt()])

            # ---------------- stage D: iFFT GEMM over m ---------------------
            with (
                tc.tile_pool(name="cs", bufs=1) as pcs,
                tc.tile_pool(name="xr", bufs=1) as pxr,
                tc.tile_pool(name="o", bufs=3) as po,
                tc.tile_pool(name="psD", bufs=2, space="PSUM") as pp2,
            ):
                csC_t, csS_t = [], []
                for mc, (ma, mb) in enumerate(MCH):
                    mp = mb - ma
                    ct = pcs.tile([mp, JH], BF16, tag=f"csC{mc}")
                    st = pcs.tile([mp, JH], BF16, tag=f"csS{mc}")
                    nc.sync.dma_start(ct[:], csC_p[ma:mb])
                    nc.sync.dma_start(st[:], csS_p[ma:mb])
                    csC_t.append(ct)
                    csS_t.append(st)

                # load the full resharded xs: [m | n*k] per (e, m-chunk)
                xr = {}
                for e in range(E):
                    for mc, (ma, mb) in enumerate(MCH):
                        mp = mb - ma
                        xt = pxr.tile([mp, N * KLAT], BF16, tag=f"xr{e}{mc}")
                        for (c, g, mlg, cnt, poff) in _dseg(ma, mb):
                            src = recvs[g][16 * c:16 * (c + 1), e,
                                           mlg:mlg + cnt, :]
                            nc.sync.dma_start(
                                xt[poff:poff + cnt].rearrange(
                                    "p (n k) -> p n k", n=N),
                                src.transpose([1, 0, 2]))
                        xr[(e, mc)] = xt

                for n in range(N):
                    for (ka, kb) in KCH:
                        kp = kb - ka
                        psA = pp2.tile([kp, JH], F32, tag="psA")
                        psB = pp2.tile([kp, JH], F32, tag="psB")
                        for mc in range(3):
                            nc.tensor.matmul(
                                psA[:],
                                xr[(0, mc)][:, n * KLAT + ka:n * KLAT + kb],
                                csC_t[mc][:],
                                start=(mc == 0), stop=(mc == 2))
                        for mc in range(3):
                            nc.tensor.matmul(
                                psB[:],
                                xr[(1, mc)][:, n * KLAT + ka:n * KLAT + kb],
                                csS_t[mc][:],
                                start=(mc == 0), stop=(mc == 2))
                        a_sb = po.tile([kp, JH], F32, tag="a_sb")
                        b_sb = po.tile([kp, JH], F32, tag="b_sb")
                        oo = po.tile([kp, NLON], F32, tag="oo")
                        nc.scalar.copy(a_sb[:], psA[:])
                        nc.scalar.copy(b_sb[:], psB[:])
                        nc.vector.tensor_tensor(
                            out=oo[:, 0:JH], in0=a_sb[:], in1=b_sb[:],
                            op=mybir.AluOpType.add)
                        nc.vector.tensor_tensor(
                            out=oo[:, JH:NLON], in0=a_sb[:, JH - 2:0:-1],
                            in1=b_sb[:, JH - 2:0:-1],
                            op=mybir.AluOpType.subtract)
                        nc.sync.dma_start(out_p[n, ka:kb], oo[:])

    if split_waits:
        _split_multi_waits(nc)
    return nc


def prep_inputs(x, sigma_n, coeff0, xi, pct):
    """Host-side shard/stage: slice + transpose per-core inputs, build constants."""
    sigma_n = np.asarray(sigma_n, np.float32)
    coeff0 = np.asarray(coeff0, np.float32)
    xi = np.asarray(xi, np.float32)
    pct = np.asarray(pct, np.float32)

    padm = MPAD - M
    padl = L2 - L
    sig_pad = np.pad(sigma_n, ((0, padl), (0, padm)))
    c0_pad = np.pad(coeff0, ((0, 0), (0, padl), (0, padm), (0, 0)))
    xi_pad = np.pad(xi, ((0, 0), (0, 0), (0, padl), (0, padm), (0, 0)))
    pct_pad = np.pad(pct, ((0, padm), (0, padl), (0, 0)))

    # half-spectrum irfft matrices (fp64 host build)
    j = np.arange(JH, dtype=np.float64)
    mm = np.arange(M, dtype=np.float64)
    ang = 2.0 * np.pi * np.outer(mm, j) / NLON
    Cm = 2.0 * np.cos(ang)
    Cm[0, :] = 1.0
    Cm[M - 1, :] = np.cos(np.pi * j)
    Sm = -2.0 * np.sin(ang)
    Sm[0, :] = 0.0
    Sm[M - 1, :] = 0.0
    Cp = np.pad(Cm, ((0, padm), (0, 0)))
    Sp = np.pad(Sm, ((0, padm), (0, 0)))

    phi_inv = PHI ** -(np.arange(T, dtype=np.float64) + 1.0)

    in_maps = []
    for c in range(NC):
        msl = slice(c * MC, (c + 1) * MC)
        # [t,n,l,m,e] -> [l,t,m,e,n], grouped into padded m-slots
        xi_c4 = np.transpose(xi_pad[:, :, :, msl, :], (2, 0, 3, 4, 1))  # l,t,m,e,n
        xi_g = np.zeros((L2, T, G, GM, E, N), np.float32)
        c0_c4 = np.transpose(c0_pad[:, :, msl, :], (1, 2, 3, 0))        # l,m,e,n
        c0_g = np.zeros((L2, G, GM, E, N), np.float32)
        sig_c = sig_pad[:, msl]                                          # l,m
        sig_g = np.zeros((L2, G, T, GM, E), np.float32)
        for g, (ga, gb) in enumerate(MGRP):
            mg = gb - ga
            xi_g[:, :, g, :mg] = xi_c4[:, :, ga:gb]
            c0_g[:, g, :mg] = c0_c4[:, ga:gb]
            sig_g[:, g, :, :mg, 0] = (sig_c[:, None, ga:gb]
                                      * phi_inv[None, :, None])
            sig_g[:, g, :, :mg, 1] = sig_g[:, g, :, :mg, 0]
        pct_c = np.ascontiguousarray(pct_pad[msl]).astype(NPBF)
        scale = FOUR_PI * PHI ** c
        in_maps.append({
            "xi_t": np.ascontiguousarray(xi_g).reshape(L2, T, MENP).astype(NPBF),
            "c0_t": np.ascontiguousarray(c0_g).reshape(L2, MENP).astype(NPBF),
            "sigp": np.ascontiguousarray(sig_g).reshape(L2, G, T, GME).astype(NPBF),
            "pct_t": pct_c,
            "csC": (scale * Cp).astype(NPBF),
            "csS": (scale * Sp).astype(NPBF),
        })
    return in_maps


_NC_CACHE = None


def kernel(x, sigma_n, coeff0, xi, pct):
    global _NC_CACHE
    in_maps = prep_inputs(x, sigma_n, coeff0, xi, pct)
    if _NC_CACHE is None:
        _NC_CACHE = build_nc()
    res = run_bass_kernel_spmd(_NC_CACHE, in_maps, list(range(NC)))
    out = np.stack([res.results[c]["out_t"] for c in range(NC)], axis=0)
    return out.reshape(T, 1, 1, N, KLAT, NLON)


# revision 17
# speedup vs baseline: 1.9635x; 1.0451x over previous
"""Trainium2 Bass kernel for CorrelatedSphericalField sampling (bf16 v4).

Math (validated against the jax reference):
  coeffs[t] = PHI^t * d_t,   d_t = d_{t-1} + PHI^{-t} * sigma_n (.) xi_{t-1},  d_0 = coeff0
  xs[t,n,k,m] = sum_l d[t,n,l,m] * pct[m,l,k]          (per-m Legendre GEMM)
  out[t,n,k,j] = 4pi * PHI^t * irfft_j(xs), as half-spectrum GEMMs:
      A[.., j] = sum_m xs_re[.., m] C[m, j],  B[.., j] = sum_m xs_im[.., m] S[m, j]
      out[.., 0:362] = A + B ;  out[.., 362+jj] = (A - B)[.., 360-jj]
  PHI^t and 4pi are folded into per-core C/S constants.

Distribution (8 cores, single launch):
  stages A+B sharded over m (46 of 368 zero-padded m's per core, all (t,n)),
  processed in 4 m-groups (16/12/10/8) pipelined with a chunked AllToAll of
  xs (shard dim = t); stage D sharded over t (core c handles t=c).

Data is bf16 end to end (fp32 PSUM accumulation, fp32 output).
"""
import numpy as np
import ml_dtypes

import concourse.bass as bass
import concourse.mybir as mybir
import concourse.tile as tile
from concourse.bass_utils import run_bass_kernel_spmd

# ---- problem constants (hardcoded; kernel must be self-contained) ----
T = 8
N = 16
L = 361          # number of degrees l (contraction dim of stage B)
L2 = 384         # L zero-padded to 3*128
KLAT = 361       # number of latitudes
M = 362          # number of orders m
NLON = 722
JH = 362         # half-spectrum output columns of stage D
NC = 8
MPAD = 368       # M padded to a multiple of NC
MC = MPAD // NC  # 46 m's per core
TN = T * N       # 128
E = 2
MEN = MC * E * N  # 1472

PHI = float(np.exp(-6.0 / 48.0))
FOUR_PI = float(4.0 * np.pi)

LCH = [(0, 128), (128, 256), (256, 384)]
MCH = [(0, 128), (128, 256), (256, 368)]
KCH = [(0, 128), (128, 256), (256, 361)]
# A2A m-chunks within a core: big first (pipeline fill), small last (tail)
MGRP = [(0, 16), (16, 28), (28, 38), (38, 46)]
G = len(MGRP)
# sigma' packed offsets: per group block [T, me_g] at SIG_OFF[g]
SIG_OFF = [T * E * ga for (ga, gb) in MGRP]

F32 = mybir.dt.float32
BF16 = mybir.dt.bfloat16
NPBF = ml_dtypes.bfloat16


def _dseg(a, b):
    """Split global-m [a,b) into (core, group, local-in-group m0, count, poff)
    segments at core (MC) and A2A-group boundaries."""
    segs = []
    mg = a
    while mg < b:
        c = mg // MC
        local = mg - c * MC
        g = next(i for i, (ga, gb) in enumerate(MGRP) if ga <= local < gb)
        hi = min(b, c * MC + MGRP[g][1])
        segs.append((c, g, local - MGRP[g][0], hi - mg, mg - a))
        mg = hi
    return segs


def _split_multi_waits(nc, max_inline=1):
    """The walrus build in this env accepts only one inline sync-wait per
    instruction; hoist extras onto same-engine NoOps placed just before."""
    ctr = 0
    for f in nc.m.functions:
        for bb in f.blocks:
            new = []
            for inst in bb.instructions:
                si = inst.sync_info
                if si is not None and si.on_wait and len(si.on_wait) > max_inline:
                    waits = list(si.on_wait)
                    keep = waits[-max_inline:]
                    for w in waits[:-max_inline]:
                        ctr += 1
                        nop = mybir.InstNoOp(name=f"I-wsplit-{ctr}",
                                             engine=inst.engine)
                        nop.sync_info = mybir.SyncInfo(on_wait=[w], on_update=[])
                        new.append(nop)
                    inst.sync_info = mybir.SyncInfo(
                        on_wait=keep, on_update=list(si.on_update))
                new.append(inst)
            bb.instructions = new


def build_nc(split_waits=True):
    nc = bass.Bass(num_devices=NC)

    # host layouts: xi [l, t, (m, e, n)], c0 [l, (m, e, n)],
    # sigp [l, group-packed (t, m_g, e)], pct [m, l(384), k]
    xi_p = nc.declare_dram_parameter("xi_t", [L2, T, MEN], BF16, isOutput=False)
    c0_p = nc.declare_dram_parameter("c0_t", [L2, MEN], BF16, isOutput=False)
    sig_p = nc.declare_dram_parameter("sigp", [L2, T * MC * E], BF16, isOutput=False)
    pct_p = nc.declare_dram_parameter("pct_t", [MC, L2, KLAT], BF16, isOutput=False)
    csC_p = nc.declare_dram_parameter("csC", [MPAD, JH], BF16, isOutput=False)
    csS_p = nc.declare_dram_parameter("csS", [MPAD, JH], BF16, isOutput=False)
    out_p = nc.declare_dram_parameter("out_t", [N, KLAT, NLON], F32, isOutput=True)

    with tile.TileContext(nc) as tc:
        with tc.tile_pool(name="dram", bufs=1, space="DRAM") as pdram:
            sends, recvs = [], []
            for g, (ga, gb) in enumerate(MGRP):
                mg = gb - ga
                sends.append(pdram.tile([TN, E, mg, KLAT], BF16,
                                        name=f"send{g}", tag=f"send{g}"))
                recvs.append(pdram.tile([TN, E, mg, KLAT], BF16,
                                        name=f"recv{g}", tag=f"recv{g}"))

            with (
                tc.tile_pool(name="per", bufs=1) as pa,
                tc.tile_pool(name="cs", bufs=1) as pcs,
                tc.tile_pool(name="xr", bufs=1) as pxr,
                tc.tile_pool(name="xi", bufs=2) as px,
                tc.tile_pool(name="w", bufs=4) as pw,
                tc.tile_pool(name="xs", bufs=4) as pxs,
                tc.tile_pool(name="psB", bufs=3, space="PSUM") as pp,
            ):
                # stage-D constants loaded up front (SP stream is in-order)
                csC_t, csS_t = [], []
                for mc, (ma, mb) in enumerate(MCH):
                    mp = mb - ma
                    ct = pcs.tile([mp, JH], BF16, name=f"csC{mc}", tag=f"csC{mc}")
                    st = pcs.tile([mp, JH], BF16, name=f"csS{mc}", tag=f"csS{mc}")
                    nc.sync.dma_start(ct[:], csC_p[ma:mb])
                    nc.sync.dma_start(st[:], csS_p[ma:mb])
                    csC_t.append(ct)
                    csS_t.append(st)
                xr = {}
                for e in range(E):
                    for mc, (ma, mb) in enumerate(MCH):
                        xr[(e, mc)] = pxr.tile([mb - ma, N * KLAT], BF16,
                                               name=f"xr{e}{mc}", tag=f"xr{e}{mc}")

                # persistent: sigma', per-(lc,g) d tiles
                sig_tiles = []
                for lc, (la, lb) in enumerate(LCH):
                    st_ = pa.tile([128, T * MC * E], BF16, tag=f"sig{lc}")
                    nc.sync.dma_start(st_[:], sig_p[la:lb])
                    sig_tiles.append(st_)
                d_tiles = {}
                for g, (ga, gb) in enumerate(MGRP):
                    me_g = (gb - ga) * E
                    for lc in range(3):
                        d_tiles[(lc, g)] = pa.tile([128, me_g, T, N], BF16,
                                                   name=f"d{lc}g{g}",
                                                   tag=f"d{lc}g{g}")

                for g, (ga, gb) in enumerate(MGRP):
                    sz = gb - ga
                    me_g = sz * E
                    men_g = me_g * N
                    # ---- stage A for group g -------------------------------
                    for lc, (la, lb) in enumerate(LCH):
                        dt_ = d_tiles[(lc, g)]
                        xi_sb = px.tile([128, T * men_g], BF16, tag="xi")
                        c0_sb = px.tile([128, men_g], BF16, tag="c0s")
                        nc.sync.dma_start(
                            xi_sb[:].rearrange("p (t q) -> p t q", t=T),
                            xi_p[la:lb, :, ga * E * N:gb * E * N])
                        nc.sync.dma_start(
                            c0_sb[:], c0_p[la:lb, ga * E * N:gb * E * N])
                        sig_b = sig_tiles[lc][
                            :, SIG_OFF[g]:SIG_OFF[g] + T * me_g][
                            :, :, None].broadcast_to([128, T * me_g, N])
                        # z = sigma' (.) xi, computed in place in xi_sb
                        z_v = xi_sb[:].rearrange("p (tq n) -> p tq n", n=N)
                        nc.vector.tensor_tensor(
                            out=z_v, in0=z_v, in1=sig_b,
                            op=mybir.AluOpType.mult)
                        nc.vector.tensor_copy(
                            dt_[:, :, 0, :],
                            c0_sb[:].rearrange("p (q n) -> p q n", n=N))
                        for t in range(1, T):
                            nc.vector.tensor_tensor(
                                out=dt_[:, :, t, :],
                                in0=dt_[:, :, t - 1, :],
                                in1=z_v[:, (t - 1) * me_g:t * me_g, :],
                                op=mybir.AluOpType.add)

                    # ---- stage B for group g -------------------------------
                    for mp_ in range(ga, gb, 2):
                        w = pw.tile([128, 2, 3, KLAT], BF16, tag="pct")
                        nc.sync.dma_start(
                            w[:],
                            pct_p[mp_:mp_ + 2].rearrange(
                                "m (c p) k -> p m c k", p=128))
                        for mi in range(2):
                            m = mp_ + mi
                            gm = m - ga
                            xs_sb = pxs.tile([TN, E, KLAT], BF16, tag="xsb")
                            for e in range(E):
                                ps = pp.tile([TN, KLAT], F32, tag=f"ps{e}")
                                for lc in range(3):
                                    nc.tensor.matmul(
                                        ps[:],
                                        d_tiles[(lc, g)][:, gm * E + e],
                                        w[:, mi, lc],
                                        start=(lc == 0), stop=(lc == 2))
                                if e == 0:
                                    nc.scalar.copy(xs_sb[:, 0], ps[:])
                                else:
                                    nc.vector.tensor_copy(xs_sb[:, 1], ps[:])
                            nc.sync.dma_start(sends[g][:, :, gm], xs_sb[:])

                    nc.gpsimd.collective_compute(
                        "AllToAll", mybir.AluOpType.bypass,
                        replica_groups=[list(range(NC))],
                        ins=[sends[g].opt()], outs=[recvs[g].opt()])

                    # xs-recv loads for THIS group (in SP order, overlap next B)
                    for e in range(E):
                        for mc, (ma, mb) in enumerate(MCH):
                            for (c, sg, mlg, cnt, poff) in _dseg(ma, mb):
                                if sg != g:
                                    continue
                                src = recvs[g][16 * c:16 * (c + 1), e,
                                               mlg:mlg + cnt, :]
                                nc.sync.dma_start(
                                    xr[(e, mc)][poff:poff + cnt].rearrange(
                                        "p (n k) -> p n k", n=N),
                                    src.transpose([1, 0, 2]))

            # ---------------- stage D: iFFT GEMM over m ---------------------
            with (
                tc.tile_pool(name="o", bufs=6) as po,
                tc.tile_pool(name="psD", bufs=3, space="PSUM") as pp2,
            ):
                for n in range(N):
                    for (ka, kb) in KCH:
                        kp = kb - ka
                        psA = pp2.tile([kp, JH], F32, tag="psA")
                        psB = pp2.tile([kp, JH], F32, tag="psB")
                        for mc in range(3):
                            nc.tensor.matmul(
                                psA[:],
                                xr[(0, mc)][:, n * KLAT + ka:n * KLAT + kb],
                                csC_t[mc][:],
                                start=(mc == 0), stop=(mc == 2))
                        for mc in range(3):
                            nc.tensor.matmul(
                                psB[:],
                                xr[(1, mc)][:, n * KLAT + ka:n * KLAT + kb],
                                csS_t[mc][:],
                                start=(mc == 0), stop=(mc == 2))
                        a_sb = po.tile([kp, JH], F32, tag="a_sb")
                        b_sb = po.tile([kp, JH], F32, tag="b_sb")
                        oo = po.tile([kp, NLON], F32, tag="oo")
                        nc.scalar.copy(a_sb[:], psA[:])
                        nc.scalar.copy(b_sb[:], psB[:])
                        nc.vector.tensor_tensor(
                            out=oo[:, 0:JH], in0=a_sb[:], in1=b_sb[:],
                            op=mybir.AluOpType.add)
                        nc.vector.tensor_tensor(
                            out=oo[:, JH:NLON], in0=a_sb[:, JH - 2:0:-1],
                            in1=b_sb[:, JH - 2:0:-1],
                            op=mybir.AluOpType.subtract)
                        nc.sync.dma_start(out_p[n, ka:kb], oo[:])

    if split_waits:
        _split_multi_waits(nc)
    return nc


def prep_inputs(x, sigma_n, coeff0, xi, pct):
    """Host-side shard/stage: slice + transpose per-core inputs, build constants."""
    sigma_n = np.asarray(sigma_n, np.float32)
    coeff0 = np.asarray(coeff0, np.float32)
    xi = np.asarray(xi, np.float32)
    pct = np.asarray(pct, np.float32)

    padm = MPAD - M
    padl = L2 - L
    sig_pad = np.pad(sigma_n, ((0, padl), (0, padm)))
    c0_pad = np.pad(coeff0, ((0, 0), (0, padl), (0, padm), (0, 0)))
    xi_pad = np.pad(xi, ((0, 0), (0, 0), (0, padl), (0, padm), (0, 0)))
    pct_pad = np.pad(pct, ((0, padm), (0, padl), (0, 0)))

    # half-spectrum irfft matrices (fp64 host build)
    j = np.arange(JH, dtype=np.float64)
    mm = np.arange(M, dtype=np.float64)
    ang = 2.0 * np.pi * np.outer(mm, j) / NLON
    Cm = 2.0 * np.cos(ang)
    Cm[0, :] = 1.0
    Cm[M - 1, :] = np.cos(np.pi * j)
    Sm = -2.0 * np.sin(ang)
    Sm[0, :] = 0.0
    Sm[M - 1, :] = 0.0
    Cp = np.pad(Cm, ((0, padm), (0, 0)))
    Sp = np.pad(Sm, ((0, padm), (0, 0)))

    phi_inv = PHI ** -(np.arange(T, dtype=np.float64) + 1.0)

    in_maps = []
    for c in range(NC):
        msl = slice(c * MC, (c + 1) * MC)
        # [t,n,l,m,e] -> [l,t,m,e,n]
        xi_c = np.ascontiguousarray(
            np.transpose(xi_pad[:, :, :, msl, :], (2, 0, 3, 4, 1))
        ).reshape(L2, T, MEN).astype(NPBF)
        # [n,l,m,e] -> [l,m,e,n]
        c0_c = np.ascontiguousarray(
            np.transpose(c0_pad[:, :, msl, :], (1, 2, 3, 0))
        ).reshape(L2, MEN).astype(NPBF)
        # sigma' group-packed: per group block [t, m_g, e]
        sig_me = (sig_pad[:, None, msl] * phi_inv[None, :, None])  # [l, t, m]
        sig_me = np.repeat(sig_me[:, :, :, None], E, axis=3)       # [l, t, m, e]
        blocks = [np.ascontiguousarray(sig_me[:, :, ga:gb]).reshape(L2, -1)
                  for (ga, gb) in MGRP]
        sig_c = np.concatenate(blocks, axis=1)
        pct_c = np.ascontiguousarray(pct_pad[msl]).astype(NPBF)
        scale = FOUR_PI * PHI ** c
        in_maps.append({
            "xi_t": xi_c,
            "c0_t": c0_c,
            "sigp": np.ascontiguousarray(sig_c).astype(NPBF),
            "pct_t": pct_c,
            "csC": (scale * Cp).astype(NPBF),
            "csS": (scale * Sp).astype(NPBF),
        })
    return in_maps


_NC_CACHE = None


def kernel(x, sigma_n, coeff0, xi, pct):
    global _NC_CACHE
    in_maps = prep_inputs(x, sigma_n, coeff0, xi, pct)
    if _NC_CACHE is None:
        _NC_CACHE = build_nc()
    res = run_bass_kernel_spmd(_NC_CACHE, in_maps, list(range(NC)))
    out = np.stack([res.results[c]["out_t"] for c in range(NC)], axis=0)
    return out.reshape(T, 1, 1, N, KLAT, NLON)
